# revision 1
# baseline (speedup 1.0000x reference)
"""BiARMA (2-layer ARMAConv GNN) Trainium2 kernel, 8-core SPMD.

Strategy: shard nodes by destination across 8 cores (12500 + pad -> 12544 each).
Host does the sharding prep (counting-sort of edges by dest tile / source block,
degree bincount -> dinv, balanced dest->tile packing). Device does all FLOPs:
  Phase A : per-node matmuls h' = dinv * (x @ W1_init), root1 = x @ W1_root + b1
  AG1     : AllGather bf16 feature table [100352, 128]
  Phase C1: per-edge dma_gather from table + one-hot matmul segment-sum in PSUM
            + per-dest dinv scale + root add  (ARMA layer 1), fused with
  Phase D : h1 transpose + layer-2 node matmuls (hh = dinv*(h1@W2_init), root2)
  AG2     : AllGather layer-2 table
  Phase C2: same edge aggregation for layer 2 -> relu -> output
All matmul inputs bf16 (fp32 PSUM accumulation); one-hot built on DVE via
broadcast is_equal against an iota row. Gather uses the MoE dma_gather ucode
(int16 idx, 4 source blocks of 25088 rows).
"""
import numpy as np
import ml_dtypes

N_CORES = 8
N_NODES = 100000
IN_CH, HID, CLS = 128, 64, 40
SHARD = 12500
SHARD_PAD = 12544          # 98 * 128
NT = SHARD_PAD // 128      # dest tiles per core
VPAD = N_CORES * SHARD_PAD # replicated table rows
BLK = 2 * SHARD_PAD        # source block rows (2 cores/shard pair), 25088 < 32767
NBLK = 4
GROUPS = [16, 16, 16, 16, 16, 16, 2]   # dest tiles per gather group (sum=98)
assert sum(GROUPS) == NT
GMAX = max(GROUPS)

bf16 = ml_dtypes.bfloat16
SUBCALL = 1024   # dma_gather ucode cap on num_idxs


def _subcalls(c_tb):
    """Sub-call (start, n) list over the token stream, shared by host wrap
    and device gather emission. Each (group, block) run is cut into <=SUBCALL
    pieces."""
    out = []
    off = 0
    for G in GROUPS:
        n = G * c_tb * 128
        for b in range(NBLK):
            done = 0
            while done < n:
                step = min(SUBCALL, n - done)
                out.append((off + done, step))
                done += step
            off += n
    return out

_PROG_CACHE = {}


# ----------------------------------------------------------------------------
# host-side prep
# ----------------------------------------------------------------------------

def _pack_tiles(cnt):
    """Greedy balanced packing of SHARD_PAD dests into NT tiles of 128.

    cnt: [SHARD_PAD, NBLK] per-dest edge counts by source block.
    Returns tile_of[d], slot_of[d] (slot in 0..127), per-(t,b) counts.
    """
    tot = cnt.sum(1)
    order = np.argsort(-tot, kind="stable")
    sums = np.zeros((NT, NBLK), np.int64)
    nd = np.zeros(NT, np.int64)
    tile_of = np.empty(SHARD_PAD, np.int32)
    slot_of = np.empty(SHARD_PAD, np.int32)
    BIG = 1 << 40
    for d in order:
        load = (sums + cnt[d]).max(axis=1) + (nd >= 128) * BIG
        t = int(np.argmin(load))
        tile_of[d] = t
        slot_of[d] = nd[t]
        nd[t] += 1
        sums[t] += cnt[d]
    return tile_of, slot_of, sums


def _prep(edge_index):
    """Full host prep. Returns per-core stream dict + C_TB."""
    row = np.asarray(edge_index[0]).astype(np.int64)
    col = np.asarray(edge_index[1]).astype(np.int64)
    deg = np.bincount(col, minlength=N_NODES).astype(np.float64)
    dinv = np.where(deg > 0, 1.0 / np.sqrt(np.maximum(deg, 1e-12)), 0.0).astype(np.float32)

    src_core = row // SHARD
    src_blk = (src_core // 2).astype(np.int64)
    dst_core = col // SHARD

    cores = []
    packs = []
    for k in range(N_CORES):
        m = dst_core == k
        er = row[m]
        ec = col[m] - k * SHARD
        eb = src_blk[m]
        cnt = np.zeros((SHARD_PAD, NBLK), np.int64)
        np.add.at(cnt, (ec, eb), 1)
        tile_of, slot_of, sums = _pack_tiles(cnt)
        packs.append((tile_of, slot_of))
        cores.append((er, ec, eb, sums))

    c_tb = max(int(np.ceil(c[3].max() / 128)) for c in cores)
    c_tb = max(c_tb, 1)
    TOT = NT * NBLK * c_tb * 128

    # global slot index per node: table row = core*SHARD_PAD + slot
    slot_global = np.empty(N_NODES, np.int64)
    perms = []   # per core: global node id in each slot (-1 for pad slots)
    for k in range(N_CORES):
        tile_of, slot_of = packs[k]
        slot_idx = tile_of * 128 + slot_of           # [SHARD_PAD] local slot of local dest d
        slot_global[k * SHARD: (k + 1) * SHARD] = slot_idx[:SHARD]
        perm = np.full(SHARD_PAD, -1, np.int64)
        d_local = np.arange(SHARD_PAD)
        node = k * SHARD + d_local
        valid = d_local < SHARD
        perm[slot_idx[valid]] = node[valid]
        perms.append(perm)

    # stream offsets: call order = for g, for b; within call: tiles of g, c_tb chunks each
    g_of_tile = np.repeat(np.arange(len(GROUPS)), GROUPS)
    tile_pos_in_g = np.concatenate([np.arange(G) for G in GROUPS])
    g_starts = np.concatenate([[0], np.cumsum(GROUPS)])[:-1]
    # token offset of call (g,b): cumulative
    call_off = {}
    off = 0
    for g, G in enumerate(GROUPS):
        for b in range(NBLK):
            call_off[(g, b)] = off
            off += G * c_tb * 128
    assert off == TOT

    streams = []
    for k in range(N_CORES):
        er, ec, eb, _sums = cores[k]
        tile_of, slot_of = packs[k]
        et = tile_of[ec]            # dest tile of each edge
        eslot = slot_of[ec]         # dest slot within tile
        # source table row (block relative)
        s_core = er // SHARD
        s_local = (s_core % 2) * SHARD_PAD + slot_global[er] % SHARD_PAD
        # base token position of (tile, block)
        base_tb = np.empty((NT, NBLK), np.int64)
        for t in range(NT):
            g = g_of_tile[t]
            for b in range(NBLK):
                base_tb[t, b] = call_off[(g, b)] + tile_pos_in_g[t] * c_tb * 128
        key = et * NBLK + eb
        order = np.argsort(key, kind="stable")
        ks = key[order]
        # rank within (t,b) group
        grp_start = np.searchsorted(ks, ks)
        rank = np.arange(len(ks)) - grp_start
        pos = base_tb[et[order], eb[order]] + rank
        tok_src = np.zeros(TOT, np.int16)
        tok_colr = np.full(TOT, 200.0, np.float32)
        tok_src[pos] = s_local[order].astype(np.int16)
        tok_colr[pos] = eslot[order].astype(np.float32)
        # idx16: per-sub-call wrap [16, n/16]
        idx16 = np.empty((16, TOT // 16), np.int16)
        for (o, n) in _subcalls(c_tb):
            idx16[:, o // 16:(o + n) // 16] = tok_src[o:o + n].reshape(n // 16, 16).T
        idx128 = np.tile(idx16, (8, 1)).copy()
        colr = tok_colr.reshape(TOT // 128, 128).T.astype(bf16).copy()
        # dinv in slot order [128, NT]
        dinv_slot = np.zeros(SHARD_PAD, np.float32)
        perm = perms[k]
        v = perm >= 0
        dinv_slot[v] = dinv[perm[v]]
        dinvT = dinv_slot.reshape(NT, 128).T.copy()
        streams.append(dict(idx=idx128, colr=colr, dinvT=dinvT, perm=perm))
    return streams, c_tb, TOT


# ----------------------------------------------------------------------------
# device program (uniform across cores; depends only on C_TB)
# ----------------------------------------------------------------------------

def _build_program(c_tb, TOT, debug=False):
    import concourse.bacc as bacc
    import concourse.mybir as mybir
    import concourse.tile as tile
    from concourse import library_config

    f32 = mybir.dt.float32
    b16 = mybir.dt.bfloat16
    i16 = mybir.dt.int16
    Copy = mybir.ActivationFunctionType.Copy
    Relu = mybir.ActivationFunctionType.Relu
    ADD = mybir.AluOpType.add
    ISEQ = mybir.AluOpType.is_equal

    nc = bacc.Bacc("TRN2", target_bir_lowering=False, debug=False, num_devices=N_CORES)

    t_x = nc.dram_tensor("x_bf", [SHARD_PAD, 128], b16, kind="ExternalInput")
    t_idx = nc.dram_tensor("idx", [128, TOT // 16], i16, kind="ExternalInput")
    t_colr = nc.dram_tensor("colr", [128, TOT // 128], b16, kind="ExternalInput")
    t_dinv = nc.dram_tensor("dinvT", [128, NT], f32, kind="ExternalInput")
    t_iota = nc.dram_tensor("iota", [128, 128], b16, kind="ExternalInput")
    t_ident = nc.dram_tensor("ident", [128, 128], f32, kind="ExternalInput")
    t_identb = nc.dram_tensor("identb", [128, 128], b16, kind="ExternalInput")
    t_w1i = nc.dram_tensor("w1i", [128, 128], b16, kind="ExternalInput")
    t_w1r = nc.dram_tensor("w1r", [128, 64], b16, kind="ExternalInput")
    t_w2i = nc.dram_tensor("w2i", [64, 128], b16, kind="ExternalInput")
    t_w2r = nc.dram_tensor("w2r", [64, 64], b16, kind="ExternalInput")
    t_b1 = nc.dram_tensor("b1b", [128, 64], f32, kind="ExternalInput")
    t_b2 = nc.dram_tensor("b2b", [128, 64], f32, kind="ExternalInput")
    t_zero = nc.dram_tensor("zeros", [128, 128], b16, kind="ExternalInput")
    t_out = nc.dram_tensor("out", [SHARD_PAD, 64], f32, kind="ExternalOutput")
    if debug:
        t_dbg_h = nc.dram_tensor("dbg_h", [SHARD_PAD, 128], b16, kind="ExternalOutput")
        t_dbg_hh = nc.dram_tensor("dbg_hh", [SHARD_PAD, 128], b16, kind="ExternalOutput")
        t_dbg_r1 = nc.dram_tensor("dbg_r1", [SHARD_PAD, 64], f32, kind="ExternalOutput")
        t_dbg_s1 = nc.dram_tensor("dbg_s1", [SHARD_PAD, 64], f32, kind="ExternalOutput")

    CHUNK_W = GMAX * c_tb * 128   # max tokens per gather call

    with tile.TileContext(nc) as tc:
        with (
            tc.tile_pool(name="cst", bufs=1) as cst,
            tc.tile_pool(name="xp", bufs=3) as xp,
            tc.tile_pool(name="hb", bufs=3) as hbp,
            tc.tile_pool(name="rootA", bufs=NT) as rootA,
            tc.tile_pool(name="rootB", bufs=NT) as rootB,
            tc.tile_pool(name="sp", bufs=4) as sp,
            tc.tile_pool(name="mp", bufs=3) as mp,
            tc.tile_pool(name="ohp", bufs=2) as ohp,
            tc.tile_pool(name="h1p", bufs=3) as h1p,
            tc.tile_pool(name="op", bufs=3) as op_,
            tc.tile_pool(name="psC", bufs=2, space="PSUM") as psC,
            tc.tile_pool(name="psD", bufs=2, space="PSUM") as psD,
            tc.tile_pool(name="psB", bufs=2, space="PSUM") as psB,
            tc.tile_pool(name="dram", bufs=1, space="DRAM") as dram,
        ):
            nc.gpsimd.load_library(library_config.mlp)

            def load_const(t, shape, dt, tag):
                s = cst.tile(shape, dt, tag=tag, name=tag)
                nc.sync.dma_start(s[:], t[:])
                return s

            iota_s = load_const(t_iota, [128, 128], b16, tag="iota_s")
            ident_s = load_const(t_ident, [128, 128], f32, tag="ident_s")
            identb_s = load_const(t_identb, [128, 128], b16, tag="identb_s")
            w1i_s = load_const(t_w1i, [128, 128], b16, tag="w1i_s")
            w1r_s = load_const(t_w1r, [128, 64], b16, tag="w1r_s")
            w2i_s = load_const(t_w2i, [64, 128], b16, tag="w2i_s")
            w2r_s = load_const(t_w2r, [64, 64], b16, tag="w2r_s")
            b1_s = load_const(t_b1, [128, 64], f32, tag="b1_s")
            b2_s = load_const(t_b2, [128, 64], f32, tag="b2_s")
            zero_s = load_const(t_zero, [128, 128], b16, tag="zero_s")
            dinv_s = load_const(t_dinv, [128, NT], f32, tag="dinv_s")
            idx_s = load_const(t_idx, [128, TOT // 16], i16, tag="idx_s")
            colr_s = load_const(t_colr, [128, TOT // 128], b16, tag="colr_s")

            h_own = dram.tile([SHARD_PAD, 128], b16)
            hh_own = dram.tile([SHARD_PAD, 128], b16)
            h_full = dram.tile([VPAD, 128], b16)
            hh_full = dram.tile([VPAD, 128], b16)

            # ---------------- Phase A ----------------
            root1 = []
            for t in range(NT):
                xa = xp.tile([128, 128], b16, tag="xa")
                nc.sync.dma_start(xa[:], t_x[t * 128:(t + 1) * 128, :])
                pX = psB.tile([128, 128], b16, tag="psB")
                nc.tensor.transpose(out=pX[:], in_=xa[:], identity=identb_s[:])
                xT = xp.tile([128, 128], b16, tag="xT")
                nc.vector.tensor_copy(out=xT[:], in_=pX[:])
                pA = psD.tile([128, 128], f32, tag="psD")
                nc.tensor.matmul(out=pA[:], lhsT=xT[:], rhs=w1i_s[:], start=True, stop=True)
                hbt = hbp.tile([128, 128], b16, tag="hb")
                nc.scalar.activation(hbt[:], pA[:], Copy, scale=dinv_s[:, t:t + 1])
                nc.sync.dma_start(h_own[t * 128:(t + 1) * 128, :], hbt[:])
                if debug:
                    nc.sync.dma_start(t_dbg_h[t * 128:(t + 1) * 128, :], hbt[:])
                pB = psB.tile([128, 128], f32, tag="psB")
                nc.tensor.matmul(out=pB[:, :64], lhsT=xT[:], rhs=w1r_s[:], start=True, stop=True)
                r1 = rootA.tile([128, 64], f32, tag="rootA")
                nc.vector.tensor_tensor(out=r1[:], in0=pB[:, :64], in1=b1_s[:], op=ADD)
                root1.append(r1)
                if debug:
                    nc.sync.dma_start(t_dbg_r1[t * 128:(t + 1) * 128, :], r1[:])

            # ---------------- AllGather 1 ----------------
            nc.gpsimd.collective_compute(
                "AllGather", mybir.AluOpType.bypass,
                replica_groups=[list(range(N_CORES))],
                ins=[h_own.opt()], outs=[h_full.opt()],
            )

            # ---------------- edge phase helper ----------------
            def edge_phase(table, readout):
                off = 0
                jg = 0
                t_base = 0
                for g, G in enumerate(GROUPS):
                    n = G * c_tb * 128
                    nch = G * c_tb
                    pC = psC.tile([128, 1024], f32, tag="psC")
                    nbanks = (G * 64 + 511) // 512
                    for bk in range(nbanks):
                        nc.tensor.matmul(
                            out=pC[:, bk * 512:(bk + 1) * 512],
                            lhsT=zero_s[:], rhs=colr_s[:, 0:512],
                            start=True, stop=False,
                        )
                    for b in range(NBLK):
                        m = mp.tile([128, CHUNK_W], b16, tag="mp")
                        done = 0
                        while done < n:
                            step = min(SUBCALL, n - done)
                            nc.gpsimd.dma_gather(
                                out_ap=m[:, done:done + step].rearrange("p (c d) -> p c d", d=128),
                                in_ap=table[b * BLK:(b + 1) * BLK, :],
                                idxs_ap=idx_s[:, (off + done) // 16:(off + done + step) // 16],
                                num_idxs=step,
                                num_idxs_reg=step,
                                elem_size=128,
                            )
                            done += step
                        oh = ohp.tile([128, CHUNK_W], b16, tag="ohp")
                        nc.vector.tensor_tensor(
                            out=oh[:, :n].rearrange("p (c d) -> p c d", d=128),
                            in0=iota_s[:].unsqueeze(1).broadcast_to([128, nch, 128]),
                            in1=colr_s[:, jg:jg + nch].unsqueeze(2).broadcast_to([128, nch, 128]),
                            op=ISEQ,
                        )
                        for ti in range(G):
                            last_of_bank = min(G - 1, (ti // 8) * 8 + 7)
                            for c in range(c_tb):
                                jj = (ti * c_tb + c) * 128
                                nc.tensor.matmul(
                                    out=pC[:, ti * 64:(ti + 1) * 64],
                                    lhsT=oh[:, jj:jj + 128],
                                    rhs=m[:, jj:jj + 64],
                                    start=False,
                                    stop=(b == NBLK - 1 and c == c_tb - 1 and ti == last_of_bank),
                                )
                        off += n
                        jg += nch
                    for ti in range(G):
                        readout(t_base + ti, pC[:, ti * 64:(ti + 1) * 64])
                    t_base += G

            # ---------------- C1 + D (layer 1 + layer-2 node compute) --------
            def readout1(t, acc):
                s1 = sp.tile([128, 64], f32, tag="s1")
                nc.scalar.activation(s1[:], acc, Copy, scale=dinv_s[:, t:t + 1])
                if debug:
                    nc.sync.dma_start(t_dbg_s1[t * 128:(t + 1) * 128, :], s1[:])
                s2 = sp.tile([128, 64], f32, tag="s2")
                nc.vector.tensor_tensor(out=s2[:], in0=s1[:], in1=root1[t][:], op=ADD)
                # D: h1T = relu(s2)^T ; hh = dinv * (h1 @ w2i) ; root2 = h1 @ w2r + b2
                pT = psB.tile([128, 128], f32, tag="psB")
                nc.tensor.transpose(out=pT[:64, :], in_=s2[:], identity=ident_s[:])
                h1t = h1p.tile([64, 128], b16, tag="h1t")
                nc.scalar.activation(h1t[:], pT[:64, :], Relu)
                pD = psD.tile([128, 128], f32, tag="psD")
                nc.tensor.matmul(out=pD[:], lhsT=h1t[:], rhs=w2i_s[:], start=True, stop=True)
                hht = hbp.tile([128, 128], b16, tag="hb")
                nc.scalar.activation(hht[:], pD[:], Copy, scale=dinv_s[:, t:t + 1])
                nc.sync.dma_start(hh_own[t * 128:(t + 1) * 128, :], hht[:])
                if debug:
                    nc.sync.dma_start(t_dbg_hh[t * 128:(t + 1) * 128, :], hht[:])
                pB2 = psB.tile([128, 128], f32, tag="psB")
                nc.tensor.matmul(out=pB2[:, :64], lhsT=h1t[:], rhs=w2r_s[:], start=True, stop=True)
                r2 = rootB.tile([128, 64], f32, tag="rootB")
                nc.vector.tensor_tensor(out=r2[:], in0=pB2[:, :64], in1=b2_s[:], op=ADD)
                root2.append(r2)

            root2 = []
            edge_phase(h_full, readout1)

            # ---------------- AllGather 2 ----------------
            nc.gpsimd.collective_compute(
                "AllGather", mybir.AluOpType.bypass,
                replica_groups=[list(range(N_CORES))],
                ins=[hh_own.opt()], outs=[hh_full.opt()],
            )

            # ---------------- C2 (layer 2 aggregation -> output) -------------
            def readout2(t, acc):
                s1 = sp.tile([128, 64], f32, tag="s1")
                nc.scalar.activation(s1[:], acc, Copy, scale=dinv_s[:, t:t + 1])
                s2 = sp.tile([128, 64], f32, tag="s2")
                nc.vector.tensor_tensor(out=s2[:], in0=s1[:], in1=root2[t][:], op=ADD)
                o = op_.tile([128, 64], f32, tag="o")
                nc.scalar.activation(o[:], s2[:], Relu)
                nc.sync.dma_start(t_out[t * 128:(t + 1) * 128, :], o[:])

            edge_phase(hh_full, readout2)

    nc.compile()
    return nc


# ----------------------------------------------------------------------------
# entry point
# ----------------------------------------------------------------------------

_LAST_RESULTS = None


def kernel(x, edge_index, w1_init, w1_root, b1, w2_init, w2_root, b2, **kw):
    global _LAST_RESULTS
    from concourse.bass_utils import run_bass_kernel_spmd

    x = np.asarray(x, np.float32)
    streams, c_tb, TOT = _prep(np.asarray(edge_index))

    import os as _os
    _dbg = _os.environ.get("BIARMA_DEBUG", "0") == "1"
    key = (c_tb, TOT, _dbg)
    if key not in _PROG_CACHE:
        _PROG_CACHE[key] = _build_program(c_tb, TOT, debug=_dbg)
    nc = _PROG_CACHE[key]

    # shared constants
    iota = np.broadcast_to(np.arange(128, dtype=np.float32), (128, 128)).astype(bf16)
    ident = np.eye(128, dtype=np.float32)
    w1i = np.zeros((128, 128), np.float32); w1i[:, :HID] = np.asarray(w1_init)
    w1r = np.asarray(w1_root, np.float32)
    w2i = np.zeros((64, 128), np.float32); w2i[:, :CLS] = np.asarray(w2_init)
    w2r = np.zeros((64, 64), np.float32); w2r[:, :CLS] = np.asarray(w2_root)
    b1b = np.broadcast_to(np.asarray(b1, np.float32), (128, HID)).copy()
    b2p = np.zeros(64, np.float32); b2p[:CLS] = np.asarray(b2)
    b2b = np.broadcast_to(b2p, (128, 64)).copy()

    in_maps = []
    for k in range(N_CORES):
        s = streams[k]
        perm = s["perm"]
        xk = np.zeros((SHARD_PAD, 128), np.float32)
        v = perm >= 0
        xk[v] = x[perm[v]]
        in_maps.append(dict(
            x_bf=xk.astype(bf16),
            idx=s["idx"],
            colr=np.asarray(s["colr"]),
            dinvT=s["dinvT"],
            iota=np.asarray(iota),
            ident=ident,
            identb=ident.astype(bf16),
            w1i=w1i.astype(bf16),
            w1r=w1r.astype(bf16),
            w2i=w2i.astype(bf16),
            w2r=w2r.astype(bf16),
            b1b=b1b,
            b2b=b2b,
            zeros=np.zeros((128, 128), bf16),
        ))

    import os
    trace = os.environ.get("BIARMA_TRACE", "0") == "1"
    try:
        res = run_bass_kernel_spmd(nc, in_maps, core_ids=list(range(N_CORES)), trace=trace)
    except ModuleNotFoundError:
        res = run_bass_kernel_spmd(nc, in_maps, core_ids=list(range(N_CORES)), trace=False)
    _LAST_RESULTS = res

    out = np.zeros((N_NODES, CLS), np.float32)
    for k in range(N_CORES):
        o = res.results[k]["out"]
        perm = streams[k]["perm"]
        v = perm >= 0
        out[perm[v]] = o[v][:, :CLS]
    return out



# revision 2
# speedup vs baseline: 13.5617x; 13.5617x over previous
"""BiARMA (2-layer ARMAConv GNN) Trainium2 kernel, 8-core SPMD — v2.

Changes vs v1:
  * Chunked AllGathers (4 slot-chunks) overlapped with edge-phase compute:
    block b of the gather reads AG chunk b, so C-phase block 0 starts as soon
    as the first chunk lands. Blocks are slot-chunks of all 8 cores
    (core-major within chunk) instead of core pairs.
  * Edge phase is block-outer / group-inner with f32 accumulation in SBUF
    (PSUM per (group, block), DVE add into acc).
  * x arrives host-pre-transposed [128, SHARD_PAD] and stays SBUF-resident;
    Phase A needs no PE transposes / DVE copies.
  * h tables are written only in cols 0:64 (the matmul never reads 64:128).
  * AllGather outputs are addr_space="Shared" (fast HBM-HBM path).
  * Tokens NOT sorted by source (random order measured faster on HW).
"""
import numpy as np
import ml_dtypes

N_CORES = 8
N_NODES = 100000
IN_CH, HID, CLS = 128, 64, 40
SHARD = 12500
SHARD_PAD = 12544          # 98 * 128
NT = SHARD_PAD // 128      # dest tiles per core (98)
VPAD = N_CORES * SHARD_PAD

# slot-chunks (AllGather chunks == gather blocks), in tiles
CHUNK_TILES = [25, 25, 25, 23]
NBLK = len(CHUNK_TILES)
CHUNK_ROWS = [t * 128 for t in CHUNK_TILES]            # per-core rows per chunk
CHUNK_START = np.concatenate([[0], np.cumsum(CHUNK_ROWS)])[:-1]
BLK_ROWS = [N_CORES * r for r in CHUNK_ROWS]           # table rows per block
BLK_START = np.concatenate([[0], np.cumsum(BLK_ROWS)])[:-1]
assert max(BLK_ROWS) <= 32767

# group split per chunk: groups are position-contiguous tile runs; psC needs
# G*64 f32 <= 2 PSUM banks -> G <= 16. Light/heavy caps experiment showed no
# viable c=2 tiles at this density, so caps are uniform.
LIGHT_PER_CHUNK = [0, 0, 0, 0]
GROUP_SPLIT = []          # list of (ntiles, cap_idx)
for _c in range(NBLK):
    L = LIGHT_PER_CHUNK[_c]
    H = CHUNK_TILES[_c] - L
    if L:
        GROUP_SPLIT.append((L, 2))
    # split heavy run into chunks of <=13 tiles (psC <= 832 f32 cols)
    while H > 0:
        take = min(13, H)
        GROUP_SPLIT.append((take, 3))
        H -= take
assert sum(g for g, _ in GROUP_SPLIT) == NT
NQUEUES = 2

bf16 = ml_dtypes.bfloat16
SUBCALL = 1024

_PROG_CACHE = {}


# ----------------------------------------------------------------------------
# host-side prep
# ----------------------------------------------------------------------------

def _pack_tiles(cnt, ntiles, dests, caps=None):
    """Greedy balanced packing of `dests` into `ntiles` tiles of 128 slots.

    cnt: [len(dests), K] per-dest edge counts by source block (K>=1).
    caps: optional [ntiles] per-block token capacity per tile; the greedy
    minimizes max-over-K of (sums+cnt)/cap (relative fill).
    Returns tile_of, slot_of (len(dests)).
    """
    tot = cnt.sum(1)
    order = np.argsort(-tot, kind="stable")
    K = cnt.shape[1]
    sums = np.zeros((ntiles, K), np.float64)
    nd = np.zeros(ntiles, np.int64)
    tile_of = np.empty(len(dests), np.int32)
    slot_of = np.empty(len(dests), np.int32)
    capv = np.ones(ntiles) if caps is None else np.asarray(caps, np.float64)
    BIG = 1 << 40
    for i in order:
        load = ((sums + cnt[i]) / capv[:, None]).max(axis=1) + (nd >= 128) * BIG
        t = int(np.argmin(load))
        tile_of[i] = t
        slot_of[i] = nd[t]
        nd[t] += 1
        sums[t] += cnt[i]
    return tile_of, slot_of


def _prep(edge_index):
    """Host prep. Returns per-core stream dict + c_tb."""
    row = np.asarray(edge_index[0]).astype(np.int64)
    col = np.asarray(edge_index[1]).astype(np.int64)
    deg = np.bincount(col, minlength=N_NODES).astype(np.float64)
    dinv = np.where(deg > 0, 1.0 / np.sqrt(np.maximum(deg, 1e-12)), 0.0).astype(np.float32)

    src_core = row // SHARD
    dst_core = col // SHARD
    chunk_of_slot = np.searchsorted(CHUNK_START, np.arange(SHARD_PAD),
                                    side="right") - 1

    # ---- pass 1: pack dests by TOTAL degree -> fixes each node's slot-CHUNK.
    # (An edge's gather block = its SOURCE node's slot-chunk; pass 2 only
    # moves dests between tiles of the same chunk, so blocks stay fixed.)
    deg_dst = np.bincount(col, minlength=N_NODES)
    chunk1 = []   # per core: chunk of each local dest [SHARD_PAD]
    for k in range(N_CORES):
        cnt = np.zeros((SHARD_PAD, 1), np.int64)
        cnt[:SHARD, 0] = deg_dst[k * SHARD:(k + 1) * SHARD]
        tile_of, _ = _pack_tiles(cnt, NT, np.arange(SHARD_PAD))
        chunk1.append(chunk_of_slot[tile_of * 128])

    # edge -> source chunk (fixed after pass 1)
    src_chunk = np.empty(len(row), np.int64)
    for k in range(N_CORES):
        m = src_core == k
        src_chunk[m] = chunk1[k][row[m] - k * SHARD]

    # ---- pass 2: within each chunk, re-pack dests by per-block counts,
    # with light tiles (first LIGHT_PER_CHUNK positions) on a smaller cap.
    packs = []
    for k in range(N_CORES):
        m = dst_core == k
        ec = col[m] - k * SHARD
        eb = src_chunk[m]
        cnt = np.zeros((SHARD_PAD, NBLK), np.int64)
        np.add.at(cnt, (ec, eb), 1)
        tile_of = np.empty(SHARD_PAD, np.int32)
        slot_of = np.empty(SHARD_PAD, np.int32)
        t0 = 0
        for c in range(NBLK):
            dests = np.nonzero(chunk1[k] == c)[0]
            assert len(dests) == CHUNK_TILES[c] * 128
            tf, sf = _pack_tiles(cnt[dests], CHUNK_TILES[c], dests)
            tile_of[dests] = t0 + tf
            slot_of[dests] = sf
            t0 += CHUNK_TILES[c]
        packs.append((tile_of, slot_of))

    # global slot of each node (pass-2 slots; chunks unchanged from pass 1)
    slot_global = np.empty(N_NODES, np.int64)
    perms = []
    for k in range(N_CORES):
        tile_of, slot_of = packs[k]
        slot_idx = tile_of * 128 + slot_of
        slot_global[k * SHARD:(k + 1) * SHARD] = slot_idx[:SHARD]
        perm = np.full(SHARD_PAD, -1, np.int64)
        d_local = np.arange(SHARD_PAD)
        node = k * SHARD + d_local
        valid = d_local < SHARD
        perm[slot_idx[valid]] = node[valid]
        perms.append(perm)

    src_slot = slot_global[row]
    # table row within block: core-major
    src_row_in_blk = src_core * np.asarray(CHUNK_ROWS)[src_chunk] + (
        src_slot - CHUNK_START[src_chunk])
    assert (src_chunk == chunk_of_slot[src_slot]).all()

    # ---- pass 2: per-core streams
    cores = []
    for k in range(N_CORES):
        m = dst_core == k
        er_row_in_blk = src_row_in_blk[m]
        eb = src_chunk[m]
        ec = col[m] - k * SHARD
        tile_of, slot_of = packs[k]
        et = tile_of[ec]
        eslot = slot_of[ec]
        cnt = np.zeros((NT, NBLK), np.int64)
        np.add.at(cnt, (et, eb), 1)
        cores.append((er_row_in_blk, eb, et, eslot, cnt))

    # resolve per-group run length from actual max (t,b) counts across cores
    maxcnt = np.zeros(NT, np.int64)
    for c in cores:
        maxcnt = np.maximum(maxcnt, c[4].max(axis=1))
    gsizes = [g for g, _ in GROUP_SPLIT]
    g_starts = np.concatenate([[0], np.cumsum(gsizes)])[:-1]
    groups = []            # resolved (ntiles, c_g)
    for gi, (G, _cap) in enumerate(GROUP_SPLIT):
        t0 = int(g_starts[gi])
        cg = max(1, int(np.ceil(maxcnt[t0:t0 + G].max() / 128)))
        groups.append((G, cg))
    groups = tuple(groups)
    TOT = NBLK * sum(G * cg * 128 for G, cg in groups)

    # token stream layout: [b][g][tiles of g][c_g*128]
    base_bt = np.empty((NBLK, NT), np.int64)
    run_of_tile = np.empty(NT, np.int64)
    off = 0
    for b in range(NBLK):
        for gi, (G, cg) in enumerate(groups):
            run = cg * 128
            for tp in range(G):
                t = int(g_starts[gi]) + tp
                base_bt[b, t] = off
                run_of_tile[t] = run
                off += run
    assert off == TOT

    streams = []
    for k in range(N_CORES):
        er_blkrow, eb, et, eslot, _cnt = cores[k]
        key = eb * NT + et
        order = np.argsort(key, kind="stable")
        ks = key[order]
        grp_start = np.searchsorted(ks, ks)
        rank = np.arange(len(ks)) - grp_start
        pos = base_bt[eb[order], et[order]] + rank
        tok_src = np.zeros(TOT, np.int16)
        tok_colr = np.full(TOT, 200.0, np.float32)
        tok_src[pos] = er_blkrow[order].astype(np.int16)
        tok_colr[pos] = eslot[order].astype(np.float32)
        # idx16 wrap: [16, TOT/16], transposed per 16-token groups
        idx16 = tok_src.reshape(TOT // 16, 16).T.copy()
        idx128 = np.tile(idx16, (8, 1)).copy()
        colr = tok_colr.reshape(TOT // 128, 128).T.astype(bf16).copy()
        # dinv in slot order [128, NT]
        perm = perms[k]
        dinv_slot = np.zeros(SHARD_PAD, np.float32)
        v = perm >= 0
        dinv_slot[v] = dinv[perm[v]]
        dinvT = dinv_slot.reshape(NT, 128).T.copy()
        streams.append(dict(idx=idx128, colr=colr, dinvT=dinvT, perm=perm))
    return streams, groups, TOT


# ----------------------------------------------------------------------------
# device program
# ----------------------------------------------------------------------------

def _build_program(groups, TOT):
    import concourse.bacc as bacc
    import concourse.mybir as mybir
    import concourse.tile as tile
    from concourse import library_config

    f32 = mybir.dt.float32
    b16 = mybir.dt.bfloat16
    i16 = mybir.dt.int16
    Copy = mybir.ActivationFunctionType.Copy
    Relu = mybir.ActivationFunctionType.Relu
    ADD = mybir.AluOpType.add
    MULT = mybir.AluOpType.mult
    ISEQ = mybir.AluOpType.is_equal

    nc = bacc.Bacc("TRN2", target_bir_lowering=False, debug=False,
                   num_devices=N_CORES, num_swdge_queues=NQUEUES)

    t_xT = nc.dram_tensor("xT", [128, SHARD_PAD], b16, kind="ExternalInput")
    t_idx = nc.dram_tensor("idx", [128, TOT // 16], i16, kind="ExternalInput")
    t_colr = nc.dram_tensor("colr", [128, TOT // 128], b16, kind="ExternalInput")
    t_dinv = nc.dram_tensor("dinvT", [128, NT], f32, kind="ExternalInput")
    t_iota = nc.dram_tensor("iota", [128, 128], b16, kind="ExternalInput")
    t_ident = nc.dram_tensor("ident", [128, 128], f32, kind="ExternalInput")
    t_w1i = nc.dram_tensor("w1i", [128, 64], b16, kind="ExternalInput")
    t_w1r = nc.dram_tensor("w1r", [128, 64], b16, kind="ExternalInput")
    t_w2i = nc.dram_tensor("w2i", [64, 64], b16, kind="ExternalInput")
    t_w2r = nc.dram_tensor("w2r", [64, 64], b16, kind="ExternalInput")
    t_b1 = nc.dram_tensor("b1b", [128, 64], f32, kind="ExternalInput")
    t_b2 = nc.dram_tensor("b2b", [128, 64], f32, kind="ExternalInput")
    t_out = nc.dram_tensor("out", [SHARD_PAD, 64], f32, kind="ExternalOutput")

    CHUNK_W = max(G * cg * 128 for G, cg in groups)   # max tokens per (g,b)
    g_starts = np.concatenate([[0], np.cumsum([g for g, _ in groups])])[:-1]

    with tile.TileContext(nc) as tc:
        with (
            tc.tile_pool(name="cst", bufs=1) as cst,
            tc.tile_pool(name="acc", bufs=1) as accp,
            tc.tile_pool(name="hb", bufs=4) as hbp,
            tc.tile_pool(name="rootA", bufs=NT) as rootA,
            tc.tile_pool(name="rootB", bufs=NT) as rootB,
            tc.tile_pool(name="sp", bufs=6) as sp,
            tc.tile_pool(name="mp", bufs=3) as mp,
            tc.tile_pool(name="ohp", bufs=2) as ohp,
            tc.tile_pool(name="h1p", bufs=4) as h1p,
            tc.tile_pool(name="op", bufs=4) as op_,
            tc.tile_pool(name="psC", bufs=1, space="PSUM") as psC,
            tc.tile_pool(name="psA", bufs=2, space="PSUM") as psA,
            tc.tile_pool(name="psR", bufs=2, space="PSUM") as psR,
            tc.tile_pool(name="dram", bufs=1, space="DRAM") as dram,
        ):
            nc.gpsimd.load_library(library_config.mlp)

            def load_const(t, shape, dt, tag):
                s = cst.tile(shape, dt, tag=tag, name=tag)
                nc.sync.dma_start(s[:], t[:])
                return s

            iota_s = load_const(t_iota, [128, 128], b16, tag="iota_s")
            ident_s = load_const(t_ident, [128, 128], f32, tag="ident_s")
            w1i_s = load_const(t_w1i, [128, 64], b16, tag="w1i_s")
            w1r_s = load_const(t_w1r, [128, 64], b16, tag="w1r_s")
            w2i_s = load_const(t_w2i, [64, 64], b16, tag="w2i_s")
            w2r_s = load_const(t_w2r, [64, 64], b16, tag="w2r_s")
            b1_s = load_const(t_b1, [128, 64], f32, tag="b1_s")
            b2_s = load_const(t_b2, [128, 64], f32, tag="b2_s")
            dinv_s = load_const(t_dinv, [128, NT], f32, tag="dinv_s")
            xT_s = load_const(t_xT, [128, SHARD_PAD], b16, tag="xT_s")
            idx_s = load_const(t_idx, [128, TOT // 16], i16, tag="idx_s")
            colr_s = load_const(t_colr, [128, TOT // 128], b16, tag="colr_s")

            h_own = dram.tile([SHARD_PAD, 128], b16)
            hh_own = dram.tile([SHARD_PAD, 128], b16)
            h_full = [dram.tile([BLK_ROWS[c], 128], b16, addr_space="Shared",
                                name=f"h_full_{c}") for c in range(NBLK)]
            hh_full = [dram.tile([BLK_ROWS[c], 128], b16, addr_space="Shared",
                                 name=f"hh_full_{c}") for c in range(NBLK)]

            # ---------------- Phase A + chunked AG1 ----------------
            root1 = []
            for c in range(NBLK):
                t0 = int(CHUNK_START[c]) // 128
                ntiles = CHUNK_TILES[c]
                for t in range(t0, t0 + ntiles):
                    lhsT = xT_s[:, t * 128:(t + 1) * 128]
                    pA = psA.tile([128, 128], f32, tag="psA")
                    nc.tensor.matmul(out=pA[:, 0:64], lhsT=lhsT, rhs=w1i_s[:],
                                     start=True, stop=True)
                    nc.tensor.matmul(out=pA[:, 64:128], lhsT=lhsT, rhs=w1r_s[:],
                                     start=True, stop=True)
                    hbt = hbp.tile([128, 64], b16, tag="hb")
                    nc.scalar.activation(hbt[:], pA[:, 0:64], Copy,
                                         scale=dinv_s[:, t:t + 1])
                    nc.sync.dma_start(h_own[t * 128:(t + 1) * 128, 0:64], hbt[:])
                    r1 = rootA.tile([128, 64], b16, tag="rootA")
                    nc.vector.tensor_tensor(out=r1[:], in0=pA[:, 64:128],
                                            in1=b1_s[:], op=ADD)
                    root1.append(r1)
                r0, r1_ = int(CHUNK_START[c]), int(CHUNK_START[c]) + CHUNK_ROWS[c]
                nc.gpsimd.collective_compute(
                    "AllGather", mybir.AluOpType.bypass,
                    replica_groups=[list(range(N_CORES))],
                    ins=[h_own[r0:r1_, :].opt()], outs=[h_full[c][:].opt()],
                )

            # ---------------- edge phase (block-outer) ----------------
            def edge_phase(table, tag):
                acc = accp.tile([128, NT * 64], f32, tag=f"acc_{tag}",
                                name=f"acc_{tag}")
                off = 0
                qi = 0
                for b in range(NBLK):
                    for g, (G, cg) in enumerate(groups):
                        n = G * cg * 128
                        nch = G * cg
                        jg = off // 128
                        m = mp.tile([128, CHUNK_W], b16, tag="mp")
                        done = 0
                        while done < n:
                            step = min(SUBCALL, n - done)
                            nc.gpsimd.dma_gather(
                                out_ap=m[:, done:done + step].rearrange(
                                    "p (c d) -> p c d", d=128),
                                in_ap=table[b][:, :],
                                idxs_ap=idx_s[:, (off + done) // 16:
                                              (off + done + step) // 16],
                                num_idxs=step,
                                num_idxs_reg=step,
                                elem_size=128,
                                queue_num=qi % NQUEUES,
                            )
                            qi += 1
                            done += step
                        oh = ohp.tile([128, CHUNK_W], b16, tag="ohp")
                        nc.vector.tensor_tensor(
                            out=oh[:, :n].rearrange("p (c d) -> p c d", d=128),
                            in0=iota_s[:].unsqueeze(1).broadcast_to([128, nch, 128]),
                            in1=colr_s[:, jg:jg + nch].unsqueeze(2).broadcast_to(
                                [128, nch, 128]),
                            op=ISEQ,
                        )
                        pC = psC.tile([128, G * 64], f32, tag="psC")
                        for ti in range(G):
                            for cc in range(cg):
                                jj = (ti * cg + cc) * 128
                                nc.tensor.matmul(
                                    out=pC[:, ti * 64:(ti + 1) * 64],
                                    lhsT=oh[:, jj:jj + 128],
                                    rhs=m[:, jj:jj + 64],
                                    start=(cc == 0),
                                    stop=(cc == cg - 1),
                                )
                        a0 = int(g_starts[g]) * 64
                        if b == 0:
                            nc.vector.tensor_copy(out=acc[:, a0:a0 + G * 64],
                                                  in_=pC[:])
                        else:
                            nc.vector.tensor_tensor(out=acc[:, a0:a0 + G * 64],
                                                    in0=acc[:, a0:a0 + G * 64],
                                                    in1=pC[:], op=ADD)
                        off += n
                return acc

            # ---------------- C1 + D + chunked AG2 ----------------
            acc1 = edge_phase(h_full, "c1")
            root2 = []
            for c in range(NBLK):
                t0 = int(CHUNK_START[c]) // 128
                for t in range(t0, t0 + CHUNK_TILES[c]):
                    s1 = sp.tile([128, 64], f32, tag="s1")
                    nc.scalar.activation(s1[:], acc1[:, t * 64:(t + 1) * 64],
                                         Copy, scale=dinv_s[:, t:t + 1])
                    s2 = sp.tile([128, 64], f32, tag="s2")
                    nc.vector.tensor_tensor(out=s2[:], in0=s1[:],
                                            in1=root1[t][:], op=ADD)
                    pT = psR.tile([128, 128], f32, tag="psT")
                    nc.tensor.transpose(out=pT[:64, :], in_=s2[:],
                                        identity=ident_s[:])
                    h1t = h1p.tile([64, 128], b16, tag="h1t")
                    nc.scalar.activation(h1t[:], pT[:64, :], Relu)
                    pDB = psR.tile([128, 128], f32, tag="psDB")
                    nc.tensor.matmul(out=pDB[:, 0:64], lhsT=h1t[:], rhs=w2i_s[:],
                                     start=True, stop=True)
                    nc.tensor.matmul(out=pDB[:, 64:128], lhsT=h1t[:], rhs=w2r_s[:],
                                     start=True, stop=True)
                    hht = hbp.tile([128, 64], b16, tag="hb2")
                    nc.scalar.activation(hht[:], pDB[:, 0:64], Copy,
                                         scale=dinv_s[:, t:t + 1])
                    nc.sync.dma_start(hh_own[t * 128:(t + 1) * 128, 0:64], hht[:])
                    r2 = rootB.tile([128, 64], b16, tag="rootB")
                    nc.vector.tensor_tensor(out=r2[:], in0=pDB[:, 64:128],
                                            in1=b2_s[:], op=ADD)
                    root2.append(r2)
                r0, r1_ = int(CHUNK_START[c]), int(CHUNK_START[c]) + CHUNK_ROWS[c]
                nc.gpsimd.collective_compute(
                    "AllGather", mybir.AluOpType.bypass,
                    replica_groups=[list(range(N_CORES))],
                    ins=[hh_own[r0:r1_, :].opt()], outs=[hh_full[c][:].opt()],
                )

            # ---------------- C2 -> output ----------------
            acc2 = edge_phase(hh_full, "c2")
            for t in range(NT):
                s1 = sp.tile([128, 64], f32, tag="s1b")
                nc.scalar.activation(s1[:], acc2[:, t * 64:(t + 1) * 64],
                                     Copy, scale=dinv_s[:, t:t + 1])
                s2 = sp.tile([128, 64], f32, tag="s2b")
                nc.vector.tensor_tensor(out=s2[:], in0=s1[:], in1=root2[t][:],
                                        op=ADD)
                o = op_.tile([128, 64], f32, tag="o")
                nc.scalar.activation(o[:], s2[:], Relu)
                nc.sync.dma_start(t_out[t * 128:(t + 1) * 128, :], o[:])

    nc.compile()
    return nc


# ----------------------------------------------------------------------------
# entry point
# ----------------------------------------------------------------------------

_LAST_RESULTS = None


def build_in_maps(inputs, streams):
    x = np.asarray(inputs["x"], np.float32)
    iota = np.broadcast_to(np.arange(128, dtype=np.float32), (128, 128)).astype(bf16)
    ident = np.eye(128, dtype=np.float32)
    w1i = np.asarray(inputs["w1_init"], np.float32)
    w1r = np.asarray(inputs["w1_root"], np.float32)
    w2i = np.zeros((64, 64), np.float32); w2i[:, :CLS] = inputs["w2_init"]
    w2r = np.zeros((64, 64), np.float32); w2r[:, :CLS] = inputs["w2_root"]
    b1b = np.broadcast_to(np.asarray(inputs["b1"], np.float32), (128, HID)).copy()
    b2p = np.zeros(64, np.float32); b2p[:CLS] = inputs["b2"]
    b2b = np.broadcast_to(b2p, (128, 64)).copy()
    in_maps = []
    for k in range(N_CORES):
        s = streams[k]
        perm = s["perm"]
        xk = np.zeros((SHARD_PAD, 128), np.float32)
        v = perm >= 0
        xk[v] = x[perm[v]]
        in_maps.append(dict(
            xT=xk.T.astype(bf16).copy(),
            idx=s["idx"], colr=np.asarray(s["colr"]), dinvT=s["dinvT"],
            iota=np.asarray(iota), ident=ident,
            w1i=w1i.astype(bf16), w1r=w1r.astype(bf16),
            w2i=w2i.astype(bf16), w2r=w2r.astype(bf16),
            b1b=b1b, b2b=b2b,
        ))
    return in_maps


def kernel(x, edge_index, w1_init, w1_root, b1, w2_init, w2_root, b2, **kw):
    global _LAST_RESULTS
    from concourse.bass_utils import run_bass_kernel_spmd

    inputs = dict(x=x, edge_index=edge_index, w1_init=w1_init, w1_root=w1_root,
                  b1=b1, w2_init=w2_init, w2_root=w2_root, b2=b2)
    streams, groups, TOT = _prep(np.asarray(edge_index))

    key = (groups, TOT)
    if key not in _PROG_CACHE:
        _PROG_CACHE[key] = _build_program(groups, TOT)
    nc = _PROG_CACHE[key]

    in_maps = build_in_maps(inputs, streams)

    import os
    trace = os.environ.get("BIARMA_TRACE", "0") == "1"
    res = run_bass_kernel_spmd(nc, in_maps, core_ids=list(range(N_CORES)),
                               trace=trace)
    _LAST_RESULTS = res

    out = np.zeros((N_NODES, CLS), np.float32)
    for k in range(N_CORES):
        o = res.results[k]["out"]
        perm = streams[k]["perm"]
        v = perm >= 0
        out[perm[v]] = o[v][:, :CLS]
    return out
